# revision 12
# baseline (speedup 1.0000x reference)
"""Trainium2 kernel for nn_AlignmentLayer.

y[l] = (x[l] - x_c[l]) @ R[l]  for l in 0..8191, x[l] is [2000, 3].

Host side computes the per-frame 3x3 rotation R[l] (Kabsch via SVD of the
64-atom cross-covariance) and translation t[l] = -x_c[l] @ R[l] -- tiny
O(L*64) work.  The device kernel does the memory-bound part: stream all of
x through SBUF and apply the per-frame affine map.

Device layout (per core, 1024 frames, data-parallel over frames):
  - frames on SBUF partitions, 128 per block, 8 blocks per core
  - each DRAM row = [12 params || 6000 coords] so one DMA per block brings
    both; params cols 0..8 = R row-major, 9..11 = t
  - compute is in-place on the x tile: for each output coord b,
      y_b = ((x_a0 * R[0,b] + t_b) + x_a1*R[1,b]) + x_a2*R[2,b]
    via tensor_scalar + 2x scalar_tensor_tensor with per-partition scalars
    and stride-3 access patterns (no deinterleave, no extra y tile)
  - raw bass with manual semaphores: SP issues all DMAs on the HWDGE FIFO
    ring, DVE does all compute; standalone wait_ge instructions only
    (this walrus build allows at most ONE attached sem wait per instruction,
    which Tile's scheduler cannot guarantee for this DMA pattern)
"""

import numpy as np

import concourse.bass as bass
import concourse.mybir as mybir
from concourse.bass_utils import run_bass_kernel_spmd

L, N, NR = 8192, 2000, 64
N_CORES = 8
L_PER_CORE = L // N_CORES          # 1024
BLOCKS = L_PER_CORE // 128         # 8
ROW = 12 + 3 * N                   # params + coords per frame
F32 = mybir.dt.float32


def _build_nc(reps=1, merge=False):
    """reps > 1 replays the whole pipeline (same data) for HW timing runs;
    all semaphore values are linear in the global block counter G.
    merge=True (reps=1 only) pairs up middle DMAs to amortize fixed costs."""
    assert not (merge and reps != 1)
    nc = bass.Bass()
    x = nc.declare_dram_parameter("x", [L_PER_CORE, ROW], F32, isOutput=False)
    y = nc.declare_dram_parameter("y", [L_PER_CORE, 3 * N], F32, isOutput=True)

    mult = mybir.AluOpType.mult
    add = mybir.AluOpType.add
    ident = mybir.ActivationFunctionType.Identity
    S = 6 if merge else 5  # x-tile slots
    TOT = BLOCKS * reps

    with (
        nc.sbuf_tensor([128, S * ROW], F32) as xts,
        nc.sbuf_tensor([128, 6 * N], F32) as tts,
        nc.semaphore("s_in") as s_in,
        nc.semaphore("s_out") as s_out,
        nc.semaphore("s_act") as s_act,
        nc.semaphore("s_dve") as s_dve,
        nc.Block() as block,
    ):
        # two sets of three t tiles, ping-ponged between ACT (producer) and
        # DVE (consumer) across blocks
        tset = [[tts[:, (3 * s + b) * N:(3 * s + b + 1) * N] for b in range(3)]
                for s in range(2)]

        def slot_ap(slot):
            return xts[:, slot * ROW:(slot + 1) * ROW]

        def out_dma(eng, G):
            eng.wait_ge(s_dve, G + 1)
            blk = G % BLOCKS
            eng.dma_start(
                out=y[blk * 128:(blk + 1) * 128, :],
                in_=xts[:, (G % S) * ROW + 12:(G % S + 1) * ROW],
            ).then_inc(s_out, 16)

        # s_in value that guarantees block G's coords are in SBUF
        if merge:
            # in-DMAs: singles for blocks 0,1 then pairs (2,3),(4,5),(6,7)
            _sin = {0: 1, 1: 2, 2: 3, 3: 3, 4: 4, 5: 4, 6: 5, 7: 5}

            def sin_val(G):
                return 16 * _sin[G]
        else:
            def sin_val(G):
                return 16 * (G + 1)

        @block.sync
        def _(sync):
            # ins only -- the SP HWDGE ring streams input blocks, gated by
            # slot-free (out complete; cross-ring so a sem is required)
            if merge:
                for blk in (0, 1):
                    sync.dma_start(
                        out=slot_ap(blk),
                        in_=x[blk * 128:(blk + 1) * 128, :],
                    ).then_inc(s_in, 16)
                for k in (1, 2, 3):
                    blk = 2 * k
                    if blk + 1 >= S:
                        # slots wrap onto 0,1: out-pair #1 (blocks 0,1) first
                        sync.wait_ge(s_out, 16)
                    sync.dma_start(
                        out=xts[:, (blk % S) * ROW:(blk % S + 2) * ROW]
                            .rearrange("p (s r) -> p s r", s=2),
                        in_=x[blk * 128:(blk + 2) * 128, :]
                            .rearrange("(s p) r -> p s r", s=2),
                    ).then_inc(s_in, 16)
            else:
                for G in range(TOT):
                    if G >= S:
                        sync.wait_ge(s_out, 16 * (G - S + 1))
                    blk = G % BLOCKS
                    sync.dma_start(
                        out=slot_ap(G % S),
                        in_=x[blk * 128:(blk + 1) * 128, :],
                    ).then_inc(s_in, 16)
            # quiesce: every stream's final count, then zero the sems --
            # hardware semaphore values persist across NEFF executions, and a
            # rerun with stale counts sails through its waits and races
            n_out_dmas = 5 if merge else TOT
            sync.wait_ge(s_in, 16 * (5 if merge else TOT))
            sync.wait_ge(s_act, TOT)
            sync.wait_ge(s_dve, TOT)
            sync.wait_ge(s_out, 16 * n_out_dmas)
            for sem in (s_in, s_act, s_dve, s_out):
                sync.sem_clear(sem)

        def out_pair(eng, blk):
            eng.wait_ge(s_dve, blk + 2)
            eng.dma_start(
                out=y[blk * 128:(blk + 2) * 128, :]
                    .rearrange("(s p) r -> p s r", s=2),
                in_=xts[:, (blk % S) * ROW:(blk % S + 2) * ROW]
                    .rearrange("p (s r) -> p s r", s=2)[:, :, 12:],
            ).then_inc(s_out, 16)

        @block.scalar
        def _(scalar):
            # ACT computes the chain heads and issues the DMA-outs on its own
            # HWDGE ring (decoupled from the in-ring)
            for G in range(TOT):
                scalar.wait_ge(s_in, sin_val(G % BLOCKS) if merge else 16 * (G + 1))
                if G >= 2:
                    # t-set reuse: DVE must be done with block G-2
                    scalar.wait_ge(s_dve, G - 1)
                xt = slot_ap(G % S)
                rt = xt[:, 0:12]
                xv = xt[:, 12:].rearrange("p (n a) -> p a n", a=3)
                ts = tset[G % 2]
                for b in range(3):
                    inst = nc.scalar.activation(
                        out=ts[b][:], in_=xv[:, 0, :], func=ident,
                        bias=rt[:, 9 + b:10 + b], scale=rt[:, b:b + 1])
                inst.then_inc(s_act, 1)
                if merge:
                    if G in (2, 4, 6):
                        out_pair(scalar, G - 2)
                elif G >= 1:
                    out_dma(scalar, G - 1)
            if merge:
                out_dma(scalar, 6)
                out_dma(scalar, 7)
            else:
                out_dma(scalar, TOT - 1)

        @block.vector
        def _(vector):
            for G in range(TOT):
                vector.wait_ge(s_in, sin_val(G % BLOCKS) if merge else 16 * (G + 1))
                vector.wait_ge(s_act, G + 1)
                xt = slot_ap(G % S)
                rt = xt[:, 0:12]
                xv = xt[:, 12:].rearrange("p (n a) -> p a n", a=3)
                ts = tset[G % 2]
                for b in range(3):
                    # in-place: t tile goes t0 -> t1
                    nc.vector.scalar_tensor_tensor(
                        out=ts[b][:], in0=xv[:, 1, :], scalar=rt[:, 3 + b:4 + b],
                        in1=ts[b][:], op0=mult, op1=add)
                for b in range(3):
                    inst = nc.vector.scalar_tensor_tensor(
                        out=xv[:, b, :], in0=xv[:, 2, :], scalar=rt[:, 6 + b:7 + b],
                        in1=ts[b][:], op0=mult, op1=add)
                inst.then_inc(s_dve, 1)
    return nc


def _host_params(x, ref_x, align_atom_indices):
    """Per-frame rotation+translation, float64 for stability -> f32."""
    idx = np.asarray(align_atom_indices).astype(np.int64)
    ref0 = np.asarray(ref_x, np.float64)
    ref0 = ref0 - ref0.mean(axis=0)
    sel = np.asarray(x[:, idx, :], np.float64)          # [L, NR, 3]
    xc = sel.mean(axis=1)                               # [L, 3]
    xn = sel - xc[:, None, :]
    prod = np.einsum("lna,nb->lab", xn, ref0)           # [L, 3, 3]
    u, s, vh = np.linalg.svd(prod)
    det = np.linalg.det(u @ vh)
    d = np.ones_like(s)
    d[:, 2] = np.sign(det)
    R = np.einsum("lij,lj,ljk->lik", u, d, vh)          # [L, 3, 3]
    t = -np.einsum("la,lab->lb", xc, R)                 # [L, 3]
    return np.concatenate([R.reshape(L, 9), t], axis=1).astype(np.float32)


def run(x, ref_x, align_atom_indices, trace=False):
    params = _host_params(x, ref_x, align_atom_indices)          # [L, 12]
    xf = np.asarray(x, np.float32).reshape(L, 3 * N)
    packed = np.concatenate([params, xf], axis=1)                # [L, ROW]
    packed = np.ascontiguousarray(packed.reshape(N_CORES, L_PER_CORE, ROW))
    # rebuild per call: bass2jax lowering mutates the module, so a cached nc
    # produces corrupt results on the second run
    nc = _build_nc()
    in_maps = [{"x": packed[i]} for i in range(N_CORES)]
    res = run_bass_kernel_spmd(nc, in_maps, core_ids=list(range(N_CORES)), trace=trace)
    out = np.concatenate([r["y"].reshape(L_PER_CORE, N, 3) for r in res.results], axis=0)
    return out, res.exec_time_ns


def kernel(x, ref_x, align_atom_indices):
    out, _ = run(x, ref_x, align_atom_indices)
    return out
